# revision 24
# baseline (speedup 1.0000x reference)
"""Trainium2 Bass kernel for nn_Decoder (Tacotron-style LSTM encoder/decoder).

Architecture (8 NeuronCores, data-parallel over batch N=64 -> 8/core):
  - Transposed ("World B") layout: hidden dim on SBUF partitions, (chunk,batch)
    on the free dim, so the h produced by the elementwise tail is directly the
    next step's matmul rhs (no per-step transposes).
  - Teacher forcing / layer chunking: input-side projections are batched into
    large matmuls per 64-step chunk; only h @ Whh.T stays per-step.
  - Cross-stream fusion: all concurrently-active LSTM streams (2 encoder
    scans, or up to 4 wavefronted decoder layers) share ONE PSUM gate tile
    and ONE fused cell-state tile, so each elementwise tail op is a single
    wide instruction instead of one per stream. Gates are reordered host-side
    to [i,i,f,f,o,o,g,g] so one sigmoid covers i+f, o is computed off the
    critical path, and tanh(g) is separate.
  - Decoder h lives in a 2-phase ring [par, slot, stream, kk, b] written by a
    single fused mul per substep; next-layer input projections and the FC
    read the previous parity's slots.
  - Weights / h / x-projections in bf16, cell state c and PSUM in fp32.
"""

import numpy as np
import ml_dtypes

H = 256
NMEL = 80
D_ENC = 512
NCORES = 8
NL = 8          # batch per core
C = 64          # chunk (time) size

_prog_cache = {}


def _build_program(S, T):
    import concourse.bass as bass
    import concourse.mybir as mybir
    import concourse.tile as tile
    from concourse import bacc
    from concourse.bass import ds
    from concourse.masks import make_identity
    from contextlib import ExitStack

    BF = mybir.dt.bfloat16
    FP = mybir.dt.float32

    TD = T
    assert S % C == 0 and TD % C == 0
    SC = S // C
    DC = TD // C

    nc = bacc.Bacc("TRN2", target_bir_lowering=False, debug=False,
                   num_devices=NCORES)

    # ---------------- DRAM I/O ----------------
    d_encrhs = nc.dram_tensor("encrhs", [4, 128, S, NL], BF, kind="ExternalInput").ap()
    d_mels = nc.dram_tensor("mels", [NL, NMEL, T], FP, kind="ExternalInput").ap()
    d_ictx = nc.dram_tensor("ictx", [NMEL], FP, kind="ExternalInput").ap()
    d_ewih = nc.dram_tensor("ewih", [128, 128, 128], BF, kind="ExternalInput").ap()
    d_ewhh = nc.dram_tensor("ewhh", [128, 64, 128], BF, kind="ExternalInput").ap()
    d_eb = nc.dram_tensor("eb", [1, 32, 128], BF, kind="ExternalInput").ap()
    d_dwih0 = nc.dram_tensor("dwih0", [97, 8, 128], BF, kind="ExternalInput").ap()
    d_dwih = nc.dram_tensor("dwih", [128, 48, 128], BF, kind="ExternalInput").ap()
    d_db = nc.dram_tensor("db", [1, 24, 128], BF, kind="ExternalInput").ap()
    d_dwhh = nc.dram_tensor("dwhh", [128, 64, 128], BF, kind="ExternalInput").ap()
    d_fcw = nc.dram_tensor("fcw", [128, 2, NMEL], BF, kind="ExternalInput").ap()
    d_fcb = nc.dram_tensor("fcb", [1, NMEL], BF, kind="ExternalInput").ap()
    d_out = nc.dram_tensor("out", [NL, NMEL, T], FP, kind="ExternalOutput").ap()
    out_r = d_out.rearrange("n c t -> c n t")  # [80, NL, T]

    CB = C * NL  # tokens per chunk = 512

    A = mybir.ActivationFunctionType

    with tile.TileContext(nc) as tc:
        with ExitStack() as ctx:
            persist = ctx.enter_context(tc.tile_pool(name="persist", bufs=1))
            psum_x = ctx.enter_context(
                tc.tile_pool(name="psx", bufs=2, space="PSUM"))
            stash = ctx.enter_context(tc.tile_pool(name="stash", bufs=2))

            ident = persist.tile([128, 128], BF)
            make_identity(nc, ident)
            ones = persist.tile([1, CB], BF)
            nc.vector.memset(ones, 1.0)

            # decoder init states copied out of encoder scope
            hinit = [persist.tile([128, 2, NL], BF, tag=f"hi{l}", name=f"hinit{l}")
                     for l in range(4)]
            cinit = [persist.tile([128, 2, NL], FP, tag=f"ci{l}", name=f"cinit{l}")
                     for l in range(4)]

            # ---------- fused substep for a group of streams ----------
            # grp dict:
            #   NS: number of stream slots; G: psum tile [128, 2, NS, 8, NL]
            #   hcur: [128, 2, NS, 2, NL] bf16 h double-buffer; substep u
            #     reads hcur[:, u] and writes hcur[:, 1-u] (all offsets are
            #     compile-time -> no per-iteration FusedRegOps on PE/DVE).
            #   sif/tg/so/tcl/t1/t2/cst: fused elementwise tiles.
            #   streams: per-stream dicts: slot, whh=(tile, base),
            #     xs(tau)->AP for identity mm (or None if grp fused_xs),
            #     hist(tau)->dest AP for the Pool history copy (or None).
            #   fused_xs(tau) -> one rhs AP covering all streams (decoder).
            def emit_substep(grp, tau, pre_tau=None):
                G = grp["G"]
                u = grp["u"]
                hcur = grp["hcur"]
                sts = grp["streams"]
                lo = min(st["slot"] for st in sts)
                hi = max(st["slot"] for st in sts)
                ns = hi - lo + 1
                r = ds(lo, ns)
                # --- PE: per-stream identity mm + whh mms (fixed APs) ---
                for st in sts:
                    s = st["slot"]
                    nc.tensor.matmul(G[:, u, s], ident, st["xcur"](u),
                                     start=True, stop=False)
                    whh, wbase = st["whh"]
                    for kk in range(2):
                        rh = hcur[:, u, s, kk, :]
                        for j in range(8):
                            nc.tensor.matmul(
                                G[:, u, s, j],
                                whh[:, wbase + kk * 8 + j, :],
                                rh, start=False, stop=(kk == 1 and j == 7))
                # --- Pool: prefetch x for substep pre_tau = tau+2 into
                # xcur[u] (same parity). Emitted AFTER the id-mm above so the
                # WAR on xcur[u] resolves this substep's read first.
                if pre_tau is not None:
                    if grp.get("fused_pre") is not None:
                        dst, srcf = grp["fused_pre"]
                        nc.gpsimd.tensor_copy(dst(u), srcf(pre_tau))
                    else:
                        for st in sts:
                            nc.gpsimd.tensor_copy(
                                st["xcur"](u), st["xpre"](pre_tau))
                # --- ACT round 1: sig(i,f) then tanh(g); sig(o) off-path ---
                sif = grp["sif"]
                tg = grp["tg"]
                so = grp["so"]
                tcl = grp["tcl"]
                cst = grp["cst"]
                t1 = grp["t1"]
                t2 = grp["t2"]
                nc.scalar.activation(sif[:, r], G[:, u, r, 0:4, :], A.Sigmoid)
                nc.scalar.activation(tg[:, r], G[:, u, r, 6:8, :], A.Tanh)
                nc.scalar.activation(so[:, r], G[:, u, r, 4:6, :], A.Sigmoid)
                # --- DVE: c update ---
                nc.vector.tensor_mul(t2[:, r], sif[:, r, 2:4, :], cst[:, r])
                nc.vector.tensor_mul(t1[:, r], sif[:, r, 0:2, :], tg[:, r])
                nc.vector.tensor_add(cst[:, r], t1[:, r], t2[:, r])
                # --- ACT round 2 + fused h write into hcur[:, 1-u] ---
                nc.scalar.activation(tcl[:, r], cst[:, r], A.Tanh)
                nc.vector.tensor_mul(hcur[:, 1 - u, r], so[:, r], tcl[:, r])
                # --- Pool: off-critical-path history copies (SBUF->SBUF) ---
                if grp.get("hist") is not None:
                    nc.gpsimd.tensor_copy(grp["hist"](tau), hcur[:, 1 - u, r])
                for st in sts:
                    if st.get("hist") is not None:
                        nc.gpsimd.tensor_copy(
                            st["hist"](tau), hcur[:, 1 - u, ds(st["slot"], 1)])

            def alloc_group(pool, pspool, NS, tagp):
                G = pspool.tile([128, 2, NS, 8, NL], FP, tag=f"{tagp}G",
                                name=f"{tagp}G")
                d = dict(NS=NS, G=G, u=0)
                d["hcur"] = pool.tile([128, 2, NS, 2, NL], BF,
                                      tag=f"{tagp}hc", name=f"{tagp}hc")
                for nm, w in [("sif", 4), ("tg", 2), ("so", 2), ("tcl", 2),
                              ("t1", 2), ("t2", 2), ("cst", 2)]:
                    d[nm] = pool.tile([128, NS, w, NL], FP,
                                      tag=f"{tagp}{nm}", name=f"{tagp}{nm}")
                return d

            # ---------- x-tilde chunk boundary (batched input projection) ----
            copy_rr = [0]

            def emit_xtilde(sc, k, par):
                """x-tilde for chunk k of scan sc into sc's xsb slot, buf par.
                sc['xdst'](par, j) gives the [128, C, NL] destination; src is
                transposed via sc['tok_tb'] if tokens are (b,t)-ordered."""
                nk = len(sc["xsrc"])
                for j in range(8):
                    xp = psum_x.tile([128, CB], FP, tag="xp")
                    for kk in range(nk):
                        rhs = sc["xsrc"][kk](k)
                        nc.tensor.matmul(xp, sc["wih"][0][:, sc["wih"][1] + kk * 8 + j, :]
                                         if sc["wih"][2] else sc["wih"][0][:, j, :],
                                         rhs, start=(kk == 0),
                                         stop=False if sc["brow"] else (kk == nk - 1))
                    if sc["brow"]:
                        nc.tensor.matmul(xp, sc["brow"][0][:, sc["brow"][1] + j, :],
                                         ones, start=False, stop=True)
                    dst = sc["xdst"](par, j)
                    src = (xp.rearrange("p (b t) -> p t b", t=C)
                           if sc.get("tok_tb") else xp)
                    rr = copy_rr[0] % 2
                    copy_rr[0] += 1
                    if rr == 0:
                        nc.scalar.copy(dst, src)
                    else:
                        nc.vector.tensor_copy(dst, src)

            # =======================================================
            # ENCODER
            # =======================================================
            with ExitStack() as ectx:
                epool = ectx.enter_context(tc.tile_pool(name="enc", bufs=1))
                psg_e = ectx.enter_context(
                    tc.tile_pool(name="psge", bufs=1, space="PSUM"))
                ew_ih = epool.tile([128, 128, 128], BF)
                ew_hh = epool.tile([128, 64, 128], BF)
                ew_b = epool.tile([1, 32, 128], BF)
                nc.sync.dma_start(out=ew_ih, in_=d_ewih)
                nc.sync.dma_start(out=ew_hh, in_=d_ewhh)
                nc.sync.dma_start(out=ew_b, in_=d_eb)

                eo_bf = epool.tile([128, 4, S, NL], BF)
                for kk in range(4):
                    nc.sync.dma_start(out=eo_bf[:, kk], in_=d_encrhs[kk])

                # fused x-tilde buffers per layer group: [par, stream, j, C, b]
                exsb = [epool.tile([128, 2, 2, 8, C, NL], BF, tag=f"exsb{l}",
                                   name=f"exsb{l}") for l in range(2)]
                # L0 h history (consumed by L1's x-tilde), global slots
                ehseq = [epool.tile([128, S + 1, 2, NL], BF, tag=f"ehs{d}",
                                    name=f"ehs{d}") for d in range(2)]

                escan = {}
                for (l, d) in [(0, 0), (0, 1), (1, 0), (1, 1)]:
                    tag = f"e{l}{d}"
                    widx = ((l * 2 + d) * 2) * 8
                    wxidx = ((l * 2 + d) * 4) * 8
                    bidx = (l * 2 + d) * 8
                    if l == 0:
                        xsrc = []
                        for kk in range(4):
                            def f(k, kk=kk, d=d):
                                tr0 = k * C if d == 0 else S - (k + 1) * C
                                return eo_bf[:, kk, tr0:tr0 + C, :]
                            xsrc.append(f)
                    else:
                        xsrc = []
                        for kk in range(4):
                            def f(k, kk=kk, d=d):
                                tr0 = k * C if d == 0 else S - (k + 1) * C
                                if kk < 2:
                                    return ehseq[0][:, tr0 + 1:tr0 + C + 1, kk, :]
                                else:
                                    return ehseq[1][:, tr0:tr0 + C, kk - 2, :]
                            xsrc.append(f)

                    def xdst(par, j, l=l, d=d):
                        return exsb[l][:, par, d, j]
                    escan[tag] = dict(
                        tag=tag, fwd=(d == 0), xdst=xdst,
                        whh=(ew_hh, widx), wih=(ew_ih, wxidx, True),
                        brow=(ew_b, bidx), xsrc=xsrc)

                xecur = epool.tile([128, 2, 2, 8, NL], BF)
                for l in range(2):
                    grp = alloc_group(epool, psg_e, 2, f"eg{l}")
                    scans = [escan[f"e{l}0"], escan[f"e{l}1"]]
                    nc.vector.memset(grp["cst"], 0.0)
                    nc.vector.memset(grp["hcur"][:, 0], 0.0)
                    for k in range(SC):
                        par = k % 2
                        for sc in scans:
                            emit_xtilde(sc, k, par)
                        streams = []
                        for si, sc in enumerate(scans):
                            t0 = k * C
                            if sc["fwd"]:
                                def xpre(tau, l=l, par=par, si=si):
                                    return exsb[l][:, par, si, :, ds(tau, 1), :]

                                def hist(tau, si=si, t0=t0):
                                    return ehseq[si][:, ds(t0 + tau + 1, 1)]
                            else:
                                def xpre(tau, l=l, par=par, si=si):
                                    # chunk stored ascending in global t; the
                                    # bwd scan walks it backward
                                    return exsb[l][:, par, si, :,
                                                   ds(C - 1 - tau, 1), :]

                                def hist(tau, si=si, t0=t0):
                                    return ehseq[si][:, ds(S - 1 - t0 - tau, 1)]

                            def xcur(u, si=si):
                                return xecur[:, u, si]
                            streams.append(dict(
                                slot=si, whh=sc["whh"], xpre=xpre, xcur=xcur,
                                hist=hist if l == 0 else None))
                        grp["streams"] = streams
                        for tau0 in range(2):  # prime xcur for tau=0,1
                            grp["u"] = tau0
                            if grp.get("fused_pre"):
                                dst, srcf = grp["fused_pre"]
                                nc.gpsimd.tensor_copy(dst(tau0), srcf(tau0))
                            else:
                                for st in streams:
                                    nc.gpsimd.tensor_copy(
                                        st["xcur"](tau0), st["xpre"](tau0))
                        with tc.For_i(0, C // 2 - 1, 1,
                                      staggered_reset=False) as i:
                            for u in range(2):
                                grp["u"] = u
                                emit_substep(grp, i * 2 + u,
                                             pre_tau=i * 2 + u + 2)
                        for tau in (C - 2, C - 1):  # peeled: no prefetch
                            grp["u"] = tau % 2
                            emit_substep(grp, tau)
                    for si in range(2):
                        nc.vector.tensor_copy(
                            hinit[2 * l + si], grp["hcur"][:, 0, si])
                        nc.vector.tensor_copy(cinit[2 * l + si],
                                              grp["cst"][:, si])

            # =======================================================
            # DECODER (4-layer chunk-lagged wavefront, fused streams)
            # =======================================================
            with ExitStack() as dctx:
                dpool = dctx.enter_context(tc.tile_pool(name="dec", bufs=1))
                psg_d = dctx.enter_context(
                    tc.tile_pool(name="psgd", bufs=1, space="PSUM"))
                dw_ih0 = dpool.tile([97, 8, 128], BF)
                dw_ih = dpool.tile([128, 48, 128], BF)
                dw_b = dpool.tile([1, 24, 128], BF)
                dw_hh = dpool.tile([128, 64, 128], BF)
                fw = dpool.tile([128, 2, NMEL], BF)
                fb = dpool.tile([1, NMEL], BF)
                nc.sync.dma_start(out=dw_ih0, in_=d_dwih0)
                nc.sync.dma_start(out=dw_ih, in_=d_dwih)
                nc.sync.dma_start(out=dw_b, in_=d_db)
                nc.sync.dma_start(out=dw_hh, in_=d_dwhh)
                nc.sync.dma_start(out=fw, in_=d_fcw)
                nc.sync.dma_start(out=fb, in_=d_fcb)

                teach = dpool.tile([97, 2, CB], BF)
                nc.vector.memset(teach, 0.0)
                nc.vector.memset(teach[96:97, :, :], 1.0)  # bias row
                mst = dpool.tile([NMEL, 2, NL, C], FP)
                icst = dpool.tile([NMEL, 1], FP)

                # fused h ring: [128, par, slot, stream, kk, b]; slot t+1
                # holds h_t of the phase with that parity (written by Pool)
                ring = dpool.tile([128, 2, C + 1, 4, 2, NL], BF)
                # fused x-tilde buffer: [par(phase), t, (stream j b) flat] so
                # one identity matmul can inject all streams' x for a substep
                dxsb = dpool.tile([128, 2, C, 4 * 8 * NL], BF)
                grp = alloc_group(dpool, psg_d, 4, "dg")

                dscan = []
                for l in range(4):
                    tag = f"d{l}"
                    if l == 0:
                        wih = (dw_ih0, 0, False)
                        brow = None
                        xsrc = [lambda k: teach[:, k % 2, :]]
                    else:
                        wih = (dw_ih, (l - 1) * 16, True)
                        brow = (dw_b, (l - 1) * 8)
                        xsrc = []
                        for kk in range(2):
                            def f(k, kk=kk, l=l):
                                # producer (l-1) wrote chunk k during phase
                                # p-1 = k+l-1 at parity (k+l-1)%2
                                par = (k + l - 1) % 2
                                return ring[:, par, 1:C + 1, l - 1, kk, :]
                            xsrc.append(f)

                    def xdst(par, j, l=l):
                        return dxsb[:, par, :, l * 64 + j * 8:l * 64 + j * 8 + 8]
                    dscan.append(dict(
                        tag=tag, xdst=xdst, whh=(dw_hh, l * 16), wih=wih,
                        brow=brow, xsrc=xsrc, tok_tb=(l == 0)))

                mels_r = d_mels.rearrange("n c t -> c n t")  # [80, NL, T]

                def fill_teacher(k):
                    t0 = k * C
                    par = k % 2
                    if k == 0:
                        nc.sync.dma_start(
                            out=icst, in_=d_ictx.rearrange("(c o) -> c o", o=1))
                        nc.vector.tensor_copy(
                            mst[:, par, :, 0], icst.to_broadcast((NMEL, NL)))
                        nc.sync.dma_start(
                            out=mst[:, par, :, 1:], in_=mels_r[:, :, 0:C - 1])
                    else:
                        nc.sync.dma_start(
                            out=mst[:, par], in_=mels_r[:, :, t0 - 1:t0 + C - 1])
                    nc.vector.tensor_copy(teach[0:NMEL, par, :],
                                          mst[:, par])

                def emit_fc(k):
                    t0 = k * C
                    par = (k + 3) % 2  # parity d3 wrote chunk k with
                    fp = psum_x.tile([NMEL, CB], FP, tag="xp", name=f"fcp{k}")
                    for kk in range(2):
                        nc.tensor.matmul(
                            fp, fw[:, kk, :],
                            ring[:, par, 1:C + 1, 3, kk, :],
                            start=(kk == 0), stop=False)
                    nc.tensor.matmul(fp, fb, ones, start=False, stop=True)
                    fst = stash.tile([NMEL, NL, C], FP, tag="fst", name=f"fst{k}")
                    nc.scalar.copy(fst, fp.rearrange("p (t b) -> p b t", b=NL))
                    if k == DC - 1:
                        nc.vector.memset(fst[:, :, C - 1:C], 0.0)
                    nc.sync.dma_start(out=out_r[:, :, t0:t0 + C], in_=fst)

                # wavefront phases
                xdcur = dpool.tile([128, 2, 4 * 8 * NL], BF)
                for p in range(DC + 3):
                    par = p % 2
                    active = [l for l in range(4) if 0 <= p - l < DC]
                    lo, hi = min(active), max(active)
                    for l in active:
                        if p - l == 0:  # newly active stream
                            nc.vector.tensor_copy(grp["hcur"][:, 0, l],
                                                  hinit[l])
                            nc.vector.tensor_copy(grp["cst"][:, l], cinit[l])
                        if l == 0:
                            fill_teacher(p)
                        emit_xtilde(dscan[l], p - l, par)
                    streams = []
                    for l in active:
                        def xcur(u, l=l):
                            return xdcur[:, u, l * 64:(l + 1) * 64]
                        streams.append(dict(slot=l, whh=dscan[l]["whh"],
                                            xcur=xcur))
                    grp["streams"] = streams

                    def pdst(u, lo=lo, hi=hi):
                        return xdcur[:, u, lo * 64:(hi + 1) * 64]

                    def psrc(tau, par=par, lo=lo, hi=hi):
                        return dxsb[:, par, ds(tau, 1), lo * 64:(hi + 1) * 64]
                    grp["fused_pre"] = (pdst, psrc)

                    def hist(tau, par=par, lo=lo, hi=hi):
                        return ring[:, par, ds(tau + 1, 1), lo:hi + 1]
                    grp["hist"] = hist
                    for tau0 in range(2):  # prime xcur for tau=0,1
                        nc.gpsimd.tensor_copy(pdst(tau0), psrc(tau0))
                    with tc.For_i(0, C // 2 - 1, 1, staggered_reset=False) as i:
                        for u in range(2):
                            grp["u"] = u
                            emit_substep(grp, i * 2 + u, pre_tau=i * 2 + u + 2)
                    for tau in (C - 2, C - 1):  # peeled: no prefetch
                        grp["u"] = tau % 2
                        emit_substep(grp, tau)
                    if 0 <= p - 4 < DC:
                        emit_fc(p - 4)

                fc_done = {p - 4 for p in range(DC + 3) if 0 <= p - 4 < DC}
                for k in range(DC):
                    if k not in fc_done:
                        emit_fc(k)

    nc.compile()
    return nc


def _host_prep(inputs):
    """Slice batch across cores + pre-tile/cast weights. Returns in_maps."""
    bf16 = ml_dtypes.bfloat16

    # gate order i,f,g,o (torch) -> i,f,o,g tile order: old j-block -> new pos
    JP = [0, 1, 2, 3, 6, 7, 4, 5]  # new j -> old j

    def tiles_T(w, kchunks, jchunks, permute):
        wT = np.ascontiguousarray(w.T).astype(bf16)  # [D, 4H]
        out = np.zeros((kchunks, jchunks, 128, 128), bf16)
        for k in range(kchunks):
            for j in range(jchunks):
                jj = JP[j] if permute else j
                out[k, j] = wT[k * 128:(k + 1) * 128, jj * 128:(jj + 1) * 128]
        return out

    def perm_b(b):
        # b: [4H] -> [8, 128] rows in permuted j order
        return np.stack([b[JP[j] * 128:(JP[j] + 1) * 128] for j in range(8)])

    enc_Wih = np.asarray(inputs["enc_Wih"], np.float32)
    enc_Whh = np.asarray(inputs["enc_Whh"], np.float32)
    enc_b = np.asarray(inputs["enc_b"], np.float32)
    dec_Wih0 = np.asarray(inputs["dec_Wih0"], np.float32)
    dec_Wih = np.asarray(inputs["dec_Wih"], np.float32)
    dec_Whh = np.asarray(inputs["dec_Whh"], np.float32)
    dec_b = np.asarray(inputs["dec_b"], np.float32)
    fc_W = np.asarray(inputs["fc_W"], np.float32)
    fc_b = np.asarray(inputs["fc_b"], np.float32)
    ictx = np.asarray(inputs["init_ctx"], np.float32).reshape(-1)

    ewih = np.zeros((2, 2, 4, 8, 128, 128), bf16)
    ewhh = np.zeros((2, 2, 2, 8, 128, 128), bf16)
    eb = np.zeros((2, 2, 8, 128), bf16)
    for l in range(2):
        for d in range(2):
            ewih[l, d] = tiles_T(enc_Wih[l, d], 4, 8, True)
            ewhh[l, d] = tiles_T(enc_Whh[l, d], 2, 8, True)
            eb[l, d] = perm_b(enc_b[l, d]).astype(bf16)

    dwih0 = np.zeros((97, 8, 128), bf16)
    w0T = dec_Wih0.T.astype(bf16)  # [80, 1024]
    for j in range(8):
        jj = JP[j]
        dwih0[0:80, j] = w0T[:, jj * 128:(jj + 1) * 128]
        dwih0[96, j] = dec_b[0, jj * 128:(jj + 1) * 128].astype(bf16)

    dwih = np.zeros((3, 2, 8, 128, 128), bf16)
    db = np.zeros((3, 8, 128), bf16)
    for l in range(3):
        dwih[l] = tiles_T(dec_Wih[l], 2, 8, True)
        db[l] = perm_b(dec_b[l + 1]).astype(bf16)
    dwhh = np.zeros((4, 2, 8, 128, 128), bf16)
    for l in range(4):
        dwhh[l] = tiles_T(dec_Whh[l], 2, 8, True)

    fcw = np.zeros((2, 128, NMEL), bf16)
    fWT = fc_W.T.astype(bf16)  # [256, 80]
    fcw[0] = fWT[0:128]
    fcw[1] = fWT[128:256]

    encout = np.asarray(inputs["encoder_outputs"], np.float32)
    mels = np.asarray(inputs["mels"], np.float32)
    N = encout.shape[0]
    nb = N // NCORES

    base = {
        "ictx": ictx,
        "ewih": np.ascontiguousarray(ewih.reshape(128, 128, 128).transpose(1, 0, 2)),
        "ewhh": np.ascontiguousarray(ewhh.reshape(64, 128, 128).transpose(1, 0, 2)),
        "eb": np.ascontiguousarray(eb.reshape(1, 32, 128)),
        "dwih0": dwih0,
        "dwih": np.ascontiguousarray(dwih.reshape(48, 128, 128).transpose(1, 0, 2)),
        "db": np.ascontiguousarray(db.reshape(1, 24, 128)),
        "dwhh": np.ascontiguousarray(dwhh.reshape(64, 128, 128).transpose(1, 0, 2)),
        "fcw": np.ascontiguousarray(fcw.transpose(1, 0, 2)),
        "fcb": fc_b.astype(bf16).reshape(1, NMEL),
    }
    S = encout.shape[1]
    in_maps = []
    for cid in range(NCORES):
        m = dict(base)
        eo = encout[cid * nb:(cid + 1) * nb]  # [nb, S, 512]
        m["encrhs"] = np.ascontiguousarray(
            eo.transpose(2, 1, 0).reshape(4, 128, S, nb).astype(bf16))
        m["mels"] = np.ascontiguousarray(mels[cid * nb:(cid + 1) * nb])
        in_maps.append(m)
    return in_maps


def kernel(encoder_outputs, mels, text_lengths, output_lengths,
           enc_Wih, enc_Whh, enc_b, dec_Wih0, dec_Wih, dec_Whh, dec_b,
           fc_W, fc_b, init_ctx):
    from concourse import bass_utils

    inputs = dict(encoder_outputs=encoder_outputs, mels=mels,
                  enc_Wih=enc_Wih, enc_Whh=enc_Whh, enc_b=enc_b,
                  dec_Wih0=dec_Wih0, dec_Wih=dec_Wih, dec_Whh=dec_Whh,
                  dec_b=dec_b, fc_W=fc_W, fc_b=fc_b, init_ctx=init_ctx)
    N, S, _ = np.asarray(encoder_outputs).shape
    T = np.asarray(mels).shape[2]
    key = (S, T)
    if key not in _prog_cache:
        _prog_cache[key] = _build_program(S, T)
    nc = _prog_cache[key]
    in_maps = _host_prep(inputs)
    res = bass_utils.run_bass_kernel_spmd(nc, in_maps, core_ids=list(range(NCORES)))
    nb = N // NCORES
    out = np.zeros((N, NMEL, T), np.float32)
    for cid in range(NCORES):
        out[cid * nb:(cid + 1) * nb] = res.results[cid]["out"]
    return (out,)


# revision 51
# speedup vs baseline: 2.7019x; 2.7019x over previous
"""Trainium2 Bass kernel for nn_Decoder (Tacotron-style LSTM encoder/decoder).

Architecture (8 NeuronCores, data-parallel over batch N=64 -> 8/core):
  - Transposed ("World B") layout: hidden dim on SBUF partitions, (chunk,batch)
    on the free dim, so the h produced by the elementwise tail is directly the
    next step's matmul rhs (no per-step transposes).
  - Teacher forcing / layer chunking: input-side projections are batched into
    large matmuls per 64-step chunk; only h @ Whh.T stays per-step.
  - Cross-stream fusion: all concurrently-active LSTM streams (2 encoder
    scans, or up to 4 wavefronted decoder layers) share ONE PSUM gate tile
    and ONE fused cell-state tile, so each elementwise tail op is a single
    wide instruction instead of one per stream. Gates are reordered host-side
    to [i,i,f,f,o,o,g,g] so one sigmoid covers i+f, o is computed off the
    critical path, and tanh(g) is separate.
  - Decoder h lives in a 2-phase ring [par, slot, stream, kk, b] written by a
    single fused mul per substep; next-layer input projections and the FC
    read the previous parity's slots.
  - Weights / h / x-projections in bf16, cell state c and PSUM in fp32.
"""

import numpy as np
import ml_dtypes

H = 256
NMEL = 80
D_ENC = 512
NCORES = 8
NL = 8          # batch per core
C = 64          # encoder chunk (time) size
CD = 32         # decoder chunk (time) size (smaller -> less wavefront lag)

_prog_cache = {}


def _build_program(S, T):
    import concourse.bass as bass
    import concourse.mybir as mybir
    import concourse.tile as tile
    from concourse import bacc
    from concourse.bass import ds
    from concourse.masks import make_identity
    from contextlib import ExitStack

    BF = mybir.dt.bfloat16
    FP = mybir.dt.float32

    TD = T
    assert S % C == 0 and TD % CD == 0
    SC = S // C
    DC = TD // CD

    nc = bacc.Bacc("TRN2", target_bir_lowering=False, debug=False,
                   num_devices=NCORES)

    # ---------------- DRAM I/O ----------------
    d_encrhs = nc.dram_tensor("encrhs", [4, 128, S, NL], BF, kind="ExternalInput").ap()
    d_mels = nc.dram_tensor("mels", [NL, NMEL, T], FP, kind="ExternalInput").ap()
    d_ictx = nc.dram_tensor("ictx", [NMEL], FP, kind="ExternalInput").ap()
    d_ewih = nc.dram_tensor("ewih", [128, 128, 128], BF, kind="ExternalInput").ap()
    d_ewhh = nc.dram_tensor("ewhh", [128, 64, 128], BF, kind="ExternalInput").ap()
    d_dwih0 = nc.dram_tensor("dwih0", [97, 8, 128], BF, kind="ExternalInput").ap()
    d_dwih = nc.dram_tensor("dwih", [128, 48, 128], BF, kind="ExternalInput").ap()
    d_dwhh = nc.dram_tensor("dwhh", [128, 64, 128], BF, kind="ExternalInput").ap()
    d_fcw = nc.dram_tensor("fcw", [128, 2, NMEL], BF, kind="ExternalInput").ap()
    d_fcb = nc.dram_tensor("fcb", [1, NMEL], BF, kind="ExternalInput").ap()
    d_ebT = nc.dram_tensor("ebT", [128, 32], FP, kind="ExternalInput").ap()
    d_dbT = nc.dram_tensor("dbT", [128, 24], FP, kind="ExternalInput").ap()
    d_out = nc.dram_tensor("out", [NL, NMEL, T], FP, kind="ExternalOutput").ap()
    out_r = d_out.rearrange("n c t -> c n t")  # [80, NL, T]

    CB = C * NL  # tokens per chunk = 512

    A = mybir.ActivationFunctionType

    with tile.TileContext(nc) as tc:
        with ExitStack() as ctx:
            persist = ctx.enter_context(tc.tile_pool(name="persist", bufs=1))
            psum_x = ctx.enter_context(
                tc.tile_pool(name="psx", bufs=2, space="PSUM"))
            stash = ctx.enter_context(tc.tile_pool(name="stash", bufs=2))

            ident = persist.tile([128, 128], BF)
            make_identity(nc, ident)
            ones = persist.tile([1, CB], BF)
            nc.vector.memset(ones, 1.0)

            # decoder init states copied out of encoder scope
            hinit = [persist.tile([128, 2, NL], BF, tag=f"hi{l}", name=f"hinit{l}")
                     for l in range(4)]
            cinit = [persist.tile([128, 2, NL], FP, tag=f"ci{l}", name=f"cinit{l}")
                     for l in range(4)]

            # ---------- fused substep for a group of streams ----------
            # grp dict:
            #   NS: number of stream slots; G: psum tile [128, 2, NS, 8, NL]
            #   hcur: [128, 2, NS, 2, NL] bf16 h double-buffer; substep u
            #     reads hcur[:, u] and writes hcur[:, 1-u] (all offsets are
            #     compile-time -> no per-iteration FusedRegOps on PE/DVE).
            #   sif/tg/so/tcl/t1/t2/cst: fused elementwise tiles.
            #   streams: per-stream dicts: slot, whh=(tile, base),
            #     xs(tau)->AP for identity mm (or None if grp fused_xs),
            #     hist(tau)->dest AP for the Pool history copy (or None).
            #   fused_xs(tau) -> one rhs AP covering all streams (decoder).
            def emit_substep(grp, tau, pre_tau=None):
                G = grp["G"]
                u = grp["u"]
                s4 = grp["s4"]  # xcur ring slot = tau % 4 (compile-time)
                hcur = grp["hcur"]
                sts = grp["streams"]
                lo = min(st["slot"] for st in sts)
                hi = max(st["slot"] for st in sts)
                ns = hi - lo + 1
                r = ds(lo, ns)
                # --- PE: per-stream identity mm + whh mms (fixed APs) ---
                for st in sts:
                    s = st["slot"]
                    nc.tensor.matmul(G[:, u, s], ident, st["xcur"](s4),
                                     start=True, stop=False)
                    whh, wbase = st["whh"]
                    for kk in range(2):
                        rh = hcur[:, u, s, kk, :]
                        for j in range(8):
                            nc.tensor.matmul(
                                G[:, u, s, j],
                                whh[:, wbase + kk * 8 + j, :],
                                rh, start=False, stop=(kk == 1 and j == 7))
                # --- Pool: prefetch x for substep pre_tau = tau+4 into the
                # same xcur ring slot (tau%4). Emitted AFTER the id-mm above
                # so the WAR on the slot resolves this substep's read first.
                if pre_tau is not None:
                    if grp.get("fused_pre") is not None:
                        dst, srcf = grp["fused_pre"]
                        nc.gpsimd.tensor_copy(dst(s4), srcf(pre_tau))
                    else:
                        for st in sts:
                            nc.gpsimd.tensor_copy(
                                st["xcur"](s4), st["xpre"](pre_tau))
                # --- ACT round 1: sig(i,f) then tanh(g); sig(o) off-path ---
                sif = grp["sif"]
                tg = grp["tg"]
                so = grp["so"]
                tcl = grp["tcl"]
                cst = grp["cst"]
                t1 = grp["t1"]
                t2 = grp["t2"]
                # tanh(g) first: t1 = sig_i*tanh_g is the long pole, and tg's
                # write-ack latency would otherwise stall it
                nc.scalar.activation(tg[:, r], G[:, u, r, 6:8, :], A.Tanh)
                nc.scalar.activation(sif[:, r], G[:, u, r, 0:4, :], A.Sigmoid)
                nc.scalar.activation(so[:, r], G[:, u, r, 4:6, :], A.Sigmoid)
                # --- DVE: c update ---
                nc.vector.tensor_mul(t1[:, r], sif[:, r, 0:2, :], tg[:, r])
                nc.vector.tensor_mul(t2[:, r], sif[:, r, 2:4, :], cst[:, r])
                nc.vector.tensor_add(cst[:, r], t1[:, r], t2[:, r])
                # --- ACT round 2 + fused h write into hcur[:, 1-u] ---
                nc.scalar.activation(tcl[:, r], cst[:, r], A.Tanh)
                nc.vector.tensor_mul(hcur[:, 1 - u, r], so[:, r], tcl[:, r])
                # --- Pool: off-critical-path history copies (SBUF->SBUF) ---
                if grp.get("hist") is not None:
                    nc.gpsimd.tensor_copy(grp["hist"](tau), hcur[:, 1 - u, r])
                for st in sts:
                    if st.get("hist") is not None:
                        nc.gpsimd.tensor_copy(
                            st["hist"](tau), hcur[:, 1 - u, ds(st["slot"], 1)])

            def alloc_group(pool, pspool, NS, tagp):
                G = pspool.tile([128, 2, NS, 8, NL], FP, tag=f"{tagp}G",
                                name=f"{tagp}G")
                d = dict(NS=NS, G=G, u=0)
                d["hcur"] = pool.tile([128, 2, NS, 2, NL], BF,
                                      tag=f"{tagp}hc", name=f"{tagp}hc")
                for nm, w in [("sif", 4), ("tg", 2), ("so", 2), ("tcl", 2),
                              ("t1", 2), ("t2", 2), ("cst", 2)]:
                    d[nm] = pool.tile([128, NS, w, NL], FP,
                                      tag=f"{tagp}{nm}", name=f"{tagp}{nm}")
                return d

            # ---------- x-tilde chunk boundary (batched input projection) ----
            copy_rr = [0]

            def emit_xtilde(sc, k, par):
                """x-tilde for chunk k of scan sc into sc's xsb slot, buf par.
                sc['xdst'](par, j) gives the [128, ct, NL] destination; src is
                transposed via sc['tok_tb'] if tokens are (b,t)-ordered.
                Bias (sc['brow'] = (biasT_tile, base)) is folded into the
                PSUM->SBUF copy as a per-partition scalar add."""
                nk = len(sc["xsrc"])
                cb = sc.get("cb", CB)
                for j in range(8):
                    xp = psum_x.tile([128, cb], FP, tag="xp",
                                     name=f"xp_{sc['tag']}_{k}_{j}")
                    for kk in range(nk):
                        rhs = sc["xsrc"][kk](k)
                        nc.tensor.matmul(xp, sc["wih"][0][:, sc["wih"][1] + kk * 8 + j, :]
                                         if sc["wih"][2] else sc["wih"][0][:, j, :],
                                         rhs, start=(kk == 0),
                                         stop=(kk == nk - 1))
                    dst = sc["xdst"](par, j)
                    src = (xp.rearrange("p (b t) -> p t b", t=sc["ct"])
                           if sc.get("tok_tb") else xp)
                    bias = (sc["brow"][0][:, sc["brow"][1] + j:sc["brow"][1] + j + 1]
                            if sc["brow"] else None)
                    rr = copy_rr[0] % 2
                    copy_rr[0] += 1
                    if rr == 0:
                        if bias is not None:
                            nc.scalar.activation(dst, src, A.Identity,
                                                 bias=bias)
                        else:
                            nc.scalar.copy(dst, src)
                    else:
                        if bias is not None:
                            nc.vector.tensor_scalar_add(dst, src, bias)
                        else:
                            nc.vector.tensor_copy(dst, src)

            # =======================================================
            # ENCODER
            # =======================================================
            with ExitStack() as ectx:
                epool = ectx.enter_context(tc.tile_pool(name="enc", bufs=1))
                psg_e = ectx.enter_context(
                    tc.tile_pool(name="psge", bufs=1, space="PSUM"))
                ew_ih = epool.tile([128, 128, 128], BF)
                ew_hh = epool.tile([128, 64, 128], BF)
                ew_bT = epool.tile([128, 32], FP)
                nc.sync.dma_start(out=ew_ih, in_=d_ewih)
                nc.sync.dma_start(out=ew_hh, in_=d_ewhh)
                nc.sync.dma_start(out=ew_bT, in_=d_ebT)

                eo_bf = epool.tile([128, 4, S, NL], BF)
                for kk in range(4):
                    nc.sync.dma_start(out=eo_bf[:, kk], in_=d_encrhs[kk])

                # fused x-tilde buffers per layer group: [par, stream, j, C, b]
                exsb = [epool.tile([128, 2, 2, 8, C, NL], BF, tag=f"exsb{l}",
                                   name=f"exsb{l}") for l in range(2)]
                # L0 h history (consumed by L1's x-tilde), global slots
                ehseq = [epool.tile([128, S + 1, 2, NL], BF, tag=f"ehs{d}",
                                    name=f"ehs{d}") for d in range(2)]

                escan = {}
                for (l, d) in [(0, 0), (0, 1), (1, 0), (1, 1)]:
                    tag = f"e{l}{d}"
                    widx = ((l * 2 + d) * 2) * 8
                    wxidx = ((l * 2 + d) * 4) * 8
                    bidx = (l * 2 + d) * 8
                    if l == 0:
                        xsrc = []
                        for kk in range(4):
                            def f(k, kk=kk, d=d):
                                tr0 = k * C if d == 0 else S - (k + 1) * C
                                return eo_bf[:, kk, tr0:tr0 + C, :]
                            xsrc.append(f)
                    else:
                        xsrc = []
                        for kk in range(4):
                            def f(k, kk=kk, d=d):
                                tr0 = k * C if d == 0 else S - (k + 1) * C
                                if kk < 2:
                                    return ehseq[0][:, tr0 + 1:tr0 + C + 1, kk, :]
                                else:
                                    return ehseq[1][:, tr0:tr0 + C, kk - 2, :]
                            xsrc.append(f)

                    def xdst(par, j, l=l, d=d):
                        return exsb[l][:, par, d, j]
                    escan[tag] = dict(
                        tag=tag, fwd=(d == 0), xdst=xdst, ct=C, cb=CB,
                        whh=(ew_hh, widx), wih=(ew_ih, wxidx, True),
                        brow=(ew_bT, bidx), xsrc=xsrc)

                xecur = epool.tile([128, 4, 2, 8, NL], BF)
                for l in range(2):
                    grp = alloc_group(epool, psg_e, 2, f"eg{l}")
                    scans = [escan[f"e{l}0"], escan[f"e{l}1"]]
                    nc.vector.memset(grp["cst"], 0.0)
                    nc.vector.memset(grp["hcur"][:, 0], 0.0)
                    for k in range(SC):
                        par = k % 2
                        for sc in scans:
                            emit_xtilde(sc, k, par)
                        streams = []
                        for si, sc in enumerate(scans):
                            t0 = k * C
                            if sc["fwd"]:
                                def xpre(tau, l=l, par=par, si=si):
                                    return exsb[l][:, par, si, :, ds(tau, 1), :]

                                def hist(tau, si=si, t0=t0):
                                    return ehseq[si][:, ds(t0 + tau + 1, 1)]
                            else:
                                def xpre(tau, l=l, par=par, si=si):
                                    # chunk stored ascending in global t; the
                                    # bwd scan walks it backward
                                    return exsb[l][:, par, si, :,
                                                   ds(C - 1 - tau, 1), :]

                                def hist(tau, si=si, t0=t0):
                                    return ehseq[si][:, ds(S - 1 - t0 - tau, 1)]

                            def xcur(s4, si=si):
                                return xecur[:, s4, si]
                            streams.append(dict(
                                slot=si, whh=sc["whh"], xpre=xpre, xcur=xcur,
                                hist=hist if l == 0 else None))
                        grp["streams"] = streams
                        for tau0 in range(4):  # prime xcur for tau=0..3
                            for st in streams:
                                nc.gpsimd.tensor_copy(
                                    st["xcur"](tau0), st["xpre"](tau0))
                        with tc.For_i(0, C // 4 - 1, 1,
                                      staggered_reset=True) as i:
                            for u in range(4):
                                grp["u"] = u % 2
                                grp["s4"] = u
                                emit_substep(grp, i * 4 + u,
                                             pre_tau=i * 4 + u + 4)
                        for tau in range(C - 4, C):  # peeled: no prefetch
                            grp["u"] = tau % 2
                            grp["s4"] = tau % 4
                            emit_substep(grp, tau)
                    for si in range(2):
                        nc.vector.tensor_copy(
                            hinit[2 * l + si], grp["hcur"][:, 0, si])
                        nc.vector.tensor_copy(cinit[2 * l + si],
                                              grp["cst"][:, si])

            # =======================================================
            # DECODER (4-layer chunk-lagged wavefront, fused streams)
            # =======================================================
            with ExitStack() as dctx:
                dpool = dctx.enter_context(tc.tile_pool(name="dec", bufs=1))
                psg_d = dctx.enter_context(
                    tc.tile_pool(name="psgd", bufs=1, space="PSUM"))
                dw_ih0 = dpool.tile([97, 8, 128], BF)
                dw_ih = dpool.tile([128, 48, 128], BF)
                dw_bT = dpool.tile([128, 24], FP)
                dw_hh = dpool.tile([128, 64, 128], BF)
                fw = dpool.tile([128, 2, NMEL], BF)
                fb = dpool.tile([1, NMEL], BF)
                nc.sync.dma_start(out=dw_ih0, in_=d_dwih0)
                nc.sync.dma_start(out=dw_ih, in_=d_dwih)
                nc.sync.dma_start(out=dw_bT, in_=d_dbT)
                nc.sync.dma_start(out=dw_hh, in_=d_dwhh)
                nc.sync.dma_start(out=fw, in_=d_fcw)
                nc.sync.dma_start(out=fb, in_=d_fcb)

                CDB = CD * NL
                teach = dpool.tile([97, 2, CDB], BF)
                nc.vector.memset(teach, 0.0)
                nc.vector.memset(teach[96:97, :, :], 1.0)  # bias row
                mst = dpool.tile([NMEL, 2, NL, CD], FP)
                icst = dpool.tile([NMEL, 1], FP)

                # fused h ring: [128, par, slot, stream, kk, b]; slot t+1
                # holds h_t of the phase with that parity (written by Pool)
                ring = dpool.tile([128, 2, CD + 1, 4, 2, NL], BF)
                # fused x-tilde buffer: [par(phase), t, (stream j b) flat]
                dxsb = dpool.tile([128, 2, CD, 4 * 8 * NL], BF)
                grp = alloc_group(dpool, psg_d, 4, "dg")

                dscan = []
                for l in range(4):
                    tag = f"d{l}"
                    if l == 0:
                        wih = (dw_ih0, 0, False)
                        brow = None
                        xsrc = [lambda k: teach[:, k % 2, :]]
                    else:
                        wih = (dw_ih, (l - 1) * 16, True)
                        brow = (dw_bT, (l - 1) * 8)
                        xsrc = []
                        for kk in range(2):
                            def f(k, kk=kk, l=l):
                                # producer (l-1) wrote chunk k during phase
                                # p-1 = k+l-1 at parity (k+l-1)%2
                                par = (k + l - 1) % 2
                                return ring[:, par, 1:CD + 1, l - 1, kk, :]
                            xsrc.append(f)

                    def xdst(par, j, l=l):
                        return dxsb[:, par, :, l * 64 + j * 8:l * 64 + j * 8 + 8]
                    dscan.append(dict(
                        tag=tag, xdst=xdst, whh=(dw_hh, l * 16), wih=wih,
                        brow=brow, xsrc=xsrc, tok_tb=(l == 0), ct=CD, cb=CDB))

                mels_r = d_mels.rearrange("n c t -> c n t")  # [80, NL, T]

                def fill_teacher(k):
                    t0 = k * CD
                    par = k % 2
                    if k == 0:
                        nc.sync.dma_start(
                            out=icst, in_=d_ictx.rearrange("(c o) -> c o", o=1))
                        nc.vector.tensor_copy(
                            mst[:, par, :, 0], icst.to_broadcast((NMEL, NL)))
                        nc.sync.dma_start(
                            out=mst[:, par, :, 1:], in_=mels_r[:, :, 0:CD - 1])
                    else:
                        nc.sync.dma_start(
                            out=mst[:, par], in_=mels_r[:, :, t0 - 1:t0 + CD - 1])
                    nc.vector.tensor_copy(teach[0:NMEL, par, :],
                                          mst[:, par])

                def emit_fc(k):
                    t0 = k * CD
                    par = (k + 3) % 2  # parity d3 wrote chunk k with
                    fp = psum_x.tile([NMEL, CDB], FP, tag="xp", name=f"fcp{k}")
                    for kk in range(2):
                        nc.tensor.matmul(
                            fp, fw[:, kk, :],
                            ring[:, par, 1:CD + 1, 3, kk, :],
                            start=(kk == 0), stop=False)
                    nc.tensor.matmul(fp, fb, ones[:, 0:CDB],
                                     start=False, stop=True)
                    fst = stash.tile([NMEL, NL, CD], FP, tag="fst", name=f"fst{k}")
                    nc.scalar.copy(fst, fp.rearrange("p (t b) -> p b t", b=NL))
                    if k == DC - 1:
                        nc.vector.memset(fst[:, :, CD - 1:CD], 0.0)
                    nc.sync.dma_start(out=out_r[:, :, t0:t0 + CD], in_=fst)

                # wavefront phases
                xdcur = dpool.tile([128, 4, 4 * 8 * NL], BF)
                for p in range(DC + 3):
                    par = p % 2
                    active = [l for l in range(4) if 0 <= p - l < DC]
                    lo, hi = min(active), max(active)
                    for l in active:
                        if p - l == 0:  # newly active stream
                            nc.vector.tensor_copy(grp["hcur"][:, 0, l],
                                                  hinit[l])
                            nc.vector.tensor_copy(grp["cst"][:, l], cinit[l])
                        if l == 0:
                            fill_teacher(p)
                        emit_xtilde(dscan[l], p - l, par)
                    streams = []
                    for l in active:
                        def xcur(s4, l=l):
                            return xdcur[:, s4, l * 64:(l + 1) * 64]
                        streams.append(dict(slot=l, whh=dscan[l]["whh"],
                                            xcur=xcur))
                    grp["streams"] = streams

                    def pdst(s4, lo=lo, hi=hi):
                        return xdcur[:, s4, lo * 64:(hi + 1) * 64]

                    def psrc(tau, par=par, lo=lo, hi=hi):
                        return dxsb[:, par, ds(tau, 1), lo * 64:(hi + 1) * 64]
                    grp["fused_pre"] = (pdst, psrc)

                    def hist(tau, par=par, lo=lo, hi=hi):
                        return ring[:, par, ds(tau + 1, 1), lo:hi + 1]
                    grp["hist"] = hist
                    for tau0 in range(4):  # prime xcur for tau=0..3
                        nc.gpsimd.tensor_copy(pdst(tau0), psrc(tau0))
                    with tc.For_i(0, CD // 4 - 1, 1, staggered_reset=True) as i:
                        for u in range(4):
                            grp["u"] = u % 2
                            grp["s4"] = u
                            emit_substep(grp, i * 4 + u, pre_tau=i * 4 + u + 4)
                    for tau in range(CD - 4, CD):  # peeled: no prefetch
                        grp["u"] = tau % 2
                        grp["s4"] = tau % 4
                        emit_substep(grp, tau)
                    if 0 <= p - 4 < DC:
                        emit_fc(p - 4)

                fc_done = {p - 4 for p in range(DC + 3) if 0 <= p - 4 < DC}
                for k in range(DC):
                    if k not in fc_done:
                        emit_fc(k)

    nc.compile()
    return nc


def _host_prep(inputs):
    """Slice batch across cores + pre-tile/cast weights. Returns in_maps."""
    bf16 = ml_dtypes.bfloat16

    # gate order i,f,g,o (torch) -> i,f,o,g tile order: old j-block -> new pos
    JP = [0, 1, 2, 3, 6, 7, 4, 5]  # new j -> old j

    def tiles_T(w, kchunks, jchunks, permute):
        wT = np.ascontiguousarray(w.T).astype(bf16)  # [D, 4H]
        out = np.zeros((kchunks, jchunks, 128, 128), bf16)
        for k in range(kchunks):
            for j in range(jchunks):
                jj = JP[j] if permute else j
                out[k, j] = wT[k * 128:(k + 1) * 128, jj * 128:(jj + 1) * 128]
        return out

    def perm_b(b):
        # b: [4H] -> [8, 128] rows in permuted j order
        return np.stack([b[JP[j] * 128:(JP[j] + 1) * 128] for j in range(8)])

    enc_Wih = np.asarray(inputs["enc_Wih"], np.float32)
    enc_Whh = np.asarray(inputs["enc_Whh"], np.float32)
    enc_b = np.asarray(inputs["enc_b"], np.float32)
    dec_Wih0 = np.asarray(inputs["dec_Wih0"], np.float32)
    dec_Wih = np.asarray(inputs["dec_Wih"], np.float32)
    dec_Whh = np.asarray(inputs["dec_Whh"], np.float32)
    dec_b = np.asarray(inputs["dec_b"], np.float32)
    fc_W = np.asarray(inputs["fc_W"], np.float32)
    fc_b = np.asarray(inputs["fc_b"], np.float32)
    ictx = np.asarray(inputs["init_ctx"], np.float32).reshape(-1)

    ewih = np.zeros((2, 2, 4, 8, 128, 128), bf16)
    ewhh = np.zeros((2, 2, 2, 8, 128, 128), bf16)
    ebT = np.zeros((128, 32), np.float32)
    for l in range(2):
        for d in range(2):
            ewih[l, d] = tiles_T(enc_Wih[l, d], 4, 8, True)
            ewhh[l, d] = tiles_T(enc_Whh[l, d], 2, 8, True)
            ebT[:, (l * 2 + d) * 8:(l * 2 + d) * 8 + 8] = \
                perm_b(enc_b[l, d]).T

    dwih0 = np.zeros((97, 8, 128), bf16)
    w0T = dec_Wih0.T.astype(bf16)  # [80, 1024]
    for j in range(8):
        jj = JP[j]
        dwih0[0:80, j] = w0T[:, jj * 128:(jj + 1) * 128]
        dwih0[96, j] = dec_b[0, jj * 128:(jj + 1) * 128].astype(bf16)

    dwih = np.zeros((3, 2, 8, 128, 128), bf16)
    dbT = np.zeros((128, 24), np.float32)
    for l in range(3):
        dwih[l] = tiles_T(dec_Wih[l], 2, 8, True)
        dbT[:, l * 8:l * 8 + 8] = perm_b(dec_b[l + 1]).T
    dwhh = np.zeros((4, 2, 8, 128, 128), bf16)
    for l in range(4):
        dwhh[l] = tiles_T(dec_Whh[l], 2, 8, True)

    fcw = np.zeros((2, 128, NMEL), bf16)
    fWT = fc_W.T.astype(bf16)  # [256, 80]
    fcw[0] = fWT[0:128]
    fcw[1] = fWT[128:256]

    encout = np.asarray(inputs["encoder_outputs"], np.float32)
    mels = np.asarray(inputs["mels"], np.float32)
    N = encout.shape[0]
    nb = N // NCORES

    base = {
        "ictx": ictx,
        "ewih": np.ascontiguousarray(ewih.reshape(128, 128, 128).transpose(1, 0, 2)),
        "ewhh": np.ascontiguousarray(ewhh.reshape(64, 128, 128).transpose(1, 0, 2)),
        "ebT": ebT,
        "dwih0": dwih0,
        "dwih": np.ascontiguousarray(dwih.reshape(48, 128, 128).transpose(1, 0, 2)),
        "dbT": dbT,
        "dwhh": np.ascontiguousarray(dwhh.reshape(64, 128, 128).transpose(1, 0, 2)),
        "fcw": np.ascontiguousarray(fcw.transpose(1, 0, 2)),
        "fcb": fc_b.astype(bf16).reshape(1, NMEL),
    }
    S = encout.shape[1]
    in_maps = []
    for cid in range(NCORES):
        m = dict(base)
        eo = encout[cid * nb:(cid + 1) * nb]  # [nb, S, 512]
        m["encrhs"] = np.ascontiguousarray(
            eo.transpose(2, 1, 0).reshape(4, 128, S, nb).astype(bf16))
        m["mels"] = np.ascontiguousarray(mels[cid * nb:(cid + 1) * nb])
        in_maps.append(m)
    return in_maps


def kernel(encoder_outputs, mels, text_lengths, output_lengths,
           enc_Wih, enc_Whh, enc_b, dec_Wih0, dec_Wih, dec_Whh, dec_b,
           fc_W, fc_b, init_ctx):
    from concourse import bass_utils

    inputs = dict(encoder_outputs=encoder_outputs, mels=mels,
                  enc_Wih=enc_Wih, enc_Whh=enc_Whh, enc_b=enc_b,
                  dec_Wih0=dec_Wih0, dec_Wih=dec_Wih, dec_Whh=dec_Whh,
                  dec_b=dec_b, fc_W=fc_W, fc_b=fc_b, init_ctx=init_ctx)
    N, S, _ = np.asarray(encoder_outputs).shape
    T = np.asarray(mels).shape[2]
    key = (S, T)
    if key not in _prog_cache:
        _prog_cache[key] = _build_program(S, T)
    nc = _prog_cache[key]
    in_maps = _host_prep(inputs)
    res = bass_utils.run_bass_kernel_spmd(nc, in_maps, core_ids=list(range(NCORES)))
    nb = N // NCORES
    out = np.zeros((N, NMEL, T), np.float32)
    for cid in range(NCORES):
        out[cid * nb:(cid + 1) * nb] = res.results[cid]["out"]
    return (out,)
